# revision 8
# baseline (speedup 1.0000x reference)
"""Distributed Trainium2 Bass kernel for a full causal attention layer.

Problem: B=2, S=2048, D_MODEL=1024, H=16, D_HEAD=64, causal + additive mask.

Sharding (8 cores): data-parallel over batch (cores 0-3 -> batch 0,
cores 4-7 -> batch 1) x tensor-parallel over heads (4 heads per core).

v2 changes vs v1 (261984 ns):
  * Head-PAIR attention chunks: score matmuls for heads {2th, 2th+1} are
    row-tiled (K=64 at partitions 0-63 / 64-127) and run CONCURRENTLY on
    the PE's 32x32 subarrays -> ~halves score matmul time.
  * One exp per (ki, q-half) covers BOTH heads via a 3D AP over a shared
    [128, 1024] PSUM pair tile -> fewer ACTIVATE fixed overheads.
  * Softmax 1/d computed as exp(-ln d) with a manually preloaded combined
    activation-table set (natural_log_exp_and_others, id 6) -> exactly ONE
    ACT_TABLE_LOAD in the whole kernel (v1 alternated exp<->reciprocal
    sets: 12 loads x ~1.3us each, stalling ScalarE and starving the PE).
  * Norm broadcast matmuls col-tiled 2-up (output partitions 0/64 of one
    PSUM bank), norms for the previous chunk emitted MID-chunk so their
    ScalarE work hides inside the attention exp stream.
  * th-major chunk order: heads 0,1 finish at ~50% and their AllToAll
    fires then; projection filler units are pumped one-per-ki between
    attention steps (emission order = engine queue order, so pumping is
    windowed to avoid head-of-line blocking on not-yet-landed data).
  * PSUM: pss pair tile 2 banks + psz 2x2 banks + pa pool 2x1 = 8 exact.
Host only transposes/shards inputs and concatenates the 8 output slices.
"""

import os
import sys

import ml_dtypes
import numpy as np

for _p in ("/opt/trn_rl_repo", "/root/.axon_site/_ro/trn_rl_repo"):
    if os.path.isdir(_p) and _p not in sys.path:
        sys.path.insert(0, _p)

import concourse.bass as bass  # noqa: E402
import concourse.mybir as mybir  # noqa: E402
from concourse import bacc  # noqa: E402
from concourse import tile  # noqa: E402
from concourse.bass_utils import run_bass_kernel_spmd  # noqa: E402

F32 = mybir.dt.float32
BF16 = mybir.dt.bfloat16

B, S, DM, H, DH = 2, 2048, 1024, 16, 64
N_CORES = 8
GROUP = 4              # cores per batch group
H_LOC = H // GROUP     # heads per core
WCOL = H_LOC * DH      # 256 projected cols per core
QR = S // GROUP        # 512 q rows owned per core after AllToAll
MASK_VAL = -1.0e5
SCALE = 1.0 / np.sqrt(DH).astype(np.float32)

DM_T = DM // 128       # 8 dmodel k-tiles
S_T = S // 128         # 16 seq 128-tiles
ACT_SET_LN_EXP = 6     # natural_log_exp_and_others in act_info.json


def build_bass():
    nc = bacc.Bacc("TRN2", target_bir_lowering=False, debug=False,
                   num_devices=N_CORES)

    xt_q = nc.dram_tensor("xt_q", [DM, S], BF16, kind="ExternalInput")
    xt_k = nc.dram_tensor("xt_k", [DM, S], BF16, kind="ExternalInput")
    xt_v = nc.dram_tensor("xt_v", [DM, S], BF16, kind="ExternalInput")
    w_q = nc.dram_tensor("w_q", [DM, WCOL], BF16, kind="ExternalInput")
    w_k = nc.dram_tensor("w_k", [DM, WCOL], BF16, kind="ExternalInput")
    w_v = nc.dram_tensor("w_v", [DM, WCOL], BF16, kind="ExternalInput")
    w_o = nc.dram_tensor("w_o", [DM, DM], BF16, kind="ExternalInput")
    bq = nc.dram_tensor("bq", [WCOL, 1], F32, kind="ExternalInput")
    bk = nc.dram_tensor("bk", [WCOL, 1], F32, kind="ExternalInput")
    bvb = nc.dram_tensor("bvb", [128, H_LOC * (DH + 1)], BF16, kind="ExternalInput")
    bob = nc.dram_tensor("bob", [128, DM], F32, kind="ExternalInput")
    maskt = nc.dram_tensor("maskt", [128, S_T], F32, kind="ExternalInput")
    tri = nc.dram_tensor("tri", [128, 128], F32, kind="ExternalInput")
    trib = nc.dram_tensor("trib", [128, 128], BF16, kind="ExternalInput")
    ones64 = nc.dram_tensor("ones64", [1, DH], BF16, kind="ExternalInput")
    out = nc.dram_tensor("out", [QR, DM], F32, kind="ExternalOutput")

    # single activation-table load, emitted BEFORE the TileContext so the
    # Tile scheduler never sees it (it wedges the scheduling sim) but it
    # still precedes every activation on the ACT queue: set 6 covers exp
    # AND ln, so the softmax exps and the ln/exp reciprocal never thrash
    # the activation-function table (v1: 12 ACT_TABLE_LOADs of ~1.3us).
    nc.scalar.add_instruction(mybir.InstLoadActFuncSet(
        name=nc.get_next_instruction_name(),
        act_func_set_id=ACT_SET_LN_EXP, ins=[], outs=[]))

    with tile.TileContext(nc) as tc:
        with (
            tc.tile_pool(name="persist", bufs=1) as pp,
            tc.tile_pool(name="xts", bufs=10) as xtp,
            tc.tile_pool(name="esb", bufs=8) as ep,
            tc.tile_pool(name="work", bufs=4) as wkp,
            tc.tile_pool(name="pa", bufs=2, space="PSUM") as pa,
            tc.tile_pool(name="pss", bufs=1, space="PSUM") as pssp,
            tc.tile_pool(name="psz", bufs=1, space="PSUM") as pszp,
            tc.tile_pool(name="dram", bufs=1, space="DRAM") as dp,
        ):
            # ---- persistent SBUF tiles ----
            wq_sb = [pp.tile([128, WCOL], BF16, tag=f"wq{i}", name=f"wq{i}") for i in range(DM_T)]
            wk_sb = [pp.tile([128, WCOL], BF16, tag=f"wk{i}", name=f"wk{i}") for i in range(DM_T)]
            wv_sb = [pp.tile([128, WCOL], BF16, tag=f"wv{i}", name=f"wv{i}") for i in range(DM_T)]
            wo_sb = [pp.tile([128, DM], BF16, tag=f"wo{i}", name=f"wo{i}") for i in range(DM_T)]
            qt_sb = [pp.tile([128, S], BF16, tag=f"qt{t}", name=f"qt{t}") for t in range(2)]
            kt_sb = [pp.tile([128, S], BF16, tag=f"kt{t}", name=f"kt{t}") for t in range(2)]
            vaug = [pp.tile([128, H_LOC * (DH + 1)], BF16, tag=f"va{k}", name=f"va{k}")
                    for k in range(S_T)]
            zt_sb = [pp.tile([128, S], BF16, tag=f"zt{t}", name=f"zt{t}") for t in range(2)]
            ztf_e = [pp.tile([128, 256], BF16, tag=f"zfe{i}", name=f"zfe{i}")
                     for i in range(N_CORES)]
            ztf_o = [pp.tile([128, 256], BF16, tag=f"zfo{i}", name=f"zfo{i}")
                     for i in range(N_CORES)]
            bq_sb = [pp.tile([128, 1], F32, tag=f"bq{t}", name=f"bq{t}") for t in range(2)]
            bk_sb = [pp.tile([128, 1], F32, tag=f"bk{t}", name=f"bk{t}") for t in range(2)]
            bvb_sb = pp.tile([128, H_LOC * (DH + 1)], BF16, tag="bvb")
            bob_sb = pp.tile([128, DM], F32, tag="bob")
            maskt_sb = pp.tile([128, S_T], F32, tag="maskt")
            trib_sb = pp.tile([128, 128], BF16, tag="trib")
            ones_sb = pp.tile([1, DH], BF16, tag="ones")
            oacc = [pp.tile([128, DM], F32, tag=f"oacc{i}", name=f"oacc{i}")
                    for i in range(4)]
            a2a_in = [dp.tile([N_CORES * 128, 256], BF16, tag=f"a2a_in{t}",
                              name=f"a2a_in{t}") for t in range(2)]
            a2a_out = [dp.tile([N_CORES * 128, 256], BF16, tag=f"a2a_out{t}",
                               name=f"a2a_out{t}") for t in range(2)]

            # ---- constants ----
            for t in range(2):
                nc.sync.dma_start(bq_sb[t], bq[128 * t:128 * (t + 1), :])
                nc.sync.dma_start(bk_sb[t], bk[128 * t:128 * (t + 1), :])
            nc.sync.dma_start(bvb_sb, bvb[:, :])
            nc.sync.dma_start(bob_sb, bob[:, :])
            nc.sync.dma_start(maskt_sb, maskt[:, :])
            nc.sync.dma_start(trib_sb, trib[:, :])
            nc.sync.dma_start(ones_sb, ones64[:, :])

            # ---------------- projection units (filler pump) ----------------
            def qk_units(xc, which):
                # which: 0 -> Q, 1 -> K. One unit per (wc, hf): an 8-deep dm
                # accumulation chain of N=512 matmuls + bias add into qt/kt.
                src_dram = (xt_q, xt_k)[which]
                w_dram = (w_q, w_k)[which]
                w_t = (wq_sb, wk_sb)[which]
                b_t = (bq_sb, bk_sb)[which]
                dst = (qt_sb, kt_sb)[which]
                xx = [None] * DM_T

                def load_x():
                    for dm in range(DM_T):
                        xx[dm] = xtp.tile([128, 1024], BF16, tag="xq", name="xq")
                        nc.sync.dma_start(
                            xx[dm],
                            src_dram[128 * dm:128 * (dm + 1),
                                     1024 * xc:1024 * (xc + 1)])
                        if xc == 0:
                            nc.sync.dma_start(
                                w_t[dm], w_dram[128 * dm:128 * (dm + 1), :])

                units = []
                for wc in range(2):
                    for hf in range(2):
                        def u(wc=wc, hf=hf):
                            if xx[0] is None:
                                load_x()
                            pq = pa.tile([128, 512], F32, tag="pa", name="pq")
                            for dm in range(DM_T):
                                nc.tensor.matmul(
                                    pq,
                                    w_t[dm][:, 128 * wc:128 * (wc + 1)],
                                    xx[dm][:, 512 * hf:512 * (hf + 1)],
                                    start=(dm == 0), stop=(dm == DM_T - 1))
                            with nc.allow_low_precision(reason="bf16 attention"):
                                nc.vector.tensor_scalar_add(
                                    dst[wc][:, 1024 * xc + 512 * hf:
                                            1024 * xc + 512 * (hf + 1)],
                                    pq, b_t[wc])
                        units.append(u)
                return units

            def v_units(xc):
                # one unit per 128-seq block: 8-deep dm chain, N=256, then
                # bias-add + ones-column into the augmented V tile
                xv_t = [None] * DM_T

                def load_x():
                    for dm in range(DM_T):
                        xv_t[dm] = xtp.tile([128, 512], BF16, tag="xv", name="xv")
                        nc.sync.dma_start(
                            xv_t[dm],
                            xt_v[128 * dm:128 * (dm + 1),
                                 512 * xc:512 * (xc + 1)])
                        if xc == 0:
                            nc.sync.dma_start(
                                wv_sb[dm], w_v[128 * dm:128 * (dm + 1), :])

                units = []
                for blk in range(4):
                    def u(blk=blk):
                        if xv_t[0] is None:
                            load_x()
                        ki = 4 * xc + blk
                        psv = pa.tile([128, 512], F32, tag="pa", name="pav")
                        for dm in range(DM_T):
                            nc.tensor.matmul(
                                psv[:, 0:WCOL],
                                xv_t[dm][:, 128 * blk:128 * (blk + 1)],
                                wv_sb[dm], start=(dm == 0), stop=(dm == DM_T - 1))
                        va3 = vaug[ki].rearrange("p (h x) -> p h x", h=H_LOC)
                        bvb3 = bvb_sb.rearrange("p (h x) -> p h x", h=H_LOC)
                        psv3 = psv[:, 0:WCOL].rearrange("p (h d) -> p h d", h=H_LOC)
                        with nc.allow_low_precision(reason="bf16 attention"):
                            nc.vector.scalar_tensor_tensor(
                                va3[:, :, 0:DH], psv3, 1.0, bvb3[:, :, 0:DH],
                                op0=mybir.AluOpType.mult, op1=mybir.AluOpType.add)
                            nc.vector.tensor_copy(
                                va3[:, :, DH:DH + 1], bvb3[:, :, DH:DH + 1])
                    units.append(u)
                return units

            def outproj_units(parity):
                # parity 0: even head-pairs (ztf_e, wo tiles 2g): accumulate
                # into oacc with bias; parity 1: odds: combine + store.
                ztf = ztf_e if parity == 0 else ztf_o
                units = []
                for bh in range(2):
                    for qt in range(2):
                        for hf in range(2):
                            def u(bh=bh, qt=qt, hf=hf):
                                pso = pa.tile([128, 512], F32, tag="pa", name="pso")
                                for g in range(4):
                                    nc.tensor.matmul(
                                        pso,
                                        ztf[4 * bh + g][:, 128 * qt:128 * (qt + 1)],
                                        wo_sb[2 * g + parity][:, 512 * hf:512 * (hf + 1)],
                                        start=(g == 0), stop=(g == 3))
                                acc = oacc[2 * bh + qt]
                                sl = slice(512 * hf, 512 * (hf + 1))
                                if parity == 0:
                                    nc.vector.tensor_add(
                                        acc[:, sl], pso, bob_sb[:, sl])
                                else:
                                    osb = wkp.tile([128, 512], F32, tag="osb")
                                    nc.vector.tensor_add(osb, pso, acc[:, sl])
                                    nc.sync.dma_start(
                                        out[256 * bh + 128 * qt:
                                            256 * bh + 128 * (qt + 1), sl],
                                        osb)
                            units.append(u)
                return units

            # ---------------- attention ----------------
            def attn_pair(th, c, mid=None):
                """Causal attention for heads {2th, 2th+1}, q chunk c (1024
                cols), scores transposed [k, q].  The two heads' score
                matmuls are row-tiled (K=64 at partitions 0/64) and run
                concurrently; one exp per (ki, q-half) covers both heads via
                a 3D AP.  Tile's list scheduler back-fills PE stalls with
                ready projection units emitted at the chunk boundaries."""
                kmax = 8 * c + 8
                psz = [pszp.tile([DH + 1, 1024], F32, tag=f"psz{hh}",
                                 name=f"psz{hh}") for hh in range(2)]
                last_ki = [8 * c + 3, kmax - 1]   # last contributor per half
                pend = []

                def emit_z(item):
                    ki, half, lo, esb = item
                    for hh in range(2):
                        nc.tensor.matmul(
                            psz[hh][:, 512 * half + lo:512 * (half + 1)],
                            vaug[ki][:, (DH + 1) * (2 * th + hh):
                                     (DH + 1) * (2 * th + hh + 1)],
                            esb[:, 512 * hh + lo:512 * (hh + 1)],
                            start=(ki == 0), stop=(ki == last_ki[half]))

                for ki in range(kmax):
                    j = ki - 8 * c
                    for half in range(2):
                        dcol = 128 * j - 512 * half
                        if j >= 0 and dcol >= 512:
                            continue          # whole half above the diagonal
                        lo = max(0, dcol) if j >= 0 else 0
                        qb = 1024 * c + 512 * half
                        pss = pssp.tile([128, 1024], F32, tag="pss", name="pss")
                        for hh in range(2):
                            nc.tensor.matmul(
                                pss[:, 512 * hh + lo:512 * (hh + 1)],
                                kt_sb[th][64 * hh:64 * (hh + 1),
                                          128 * ki:128 * (ki + 1)],
                                qt_sb[th][64 * hh:64 * (hh + 1),
                                          qb + lo:qb + 512],
                                start=True, stop=True)
                        esb = ep.tile([128, 1024], BF16, tag="e", name="esb")
                        p3 = pss.rearrange("p (h w) -> p h w", h=2)
                        e3 = esb.rearrange("p (h w) -> p h w", h=2)
                        nc.scalar.activation(
                            e3[:, :, lo:512], p3[:, :, lo:512],
                            mybir.ActivationFunctionType.Exp,
                            bias=maskt_sb[:, ki:ki + 1], scale=float(SCALE))
                        if j >= 0 and dcol >= 0:
                            # diagonal: post-exp 0/1 triangle mask per head
                            with nc.allow_low_precision(reason="bf16 attention"):
                                for hh in range(2):
                                    nc.vector.tensor_mul(
                                        esb[:, 512 * hh + lo:512 * hh + lo + 128],
                                        esb[:, 512 * hh + lo:512 * hh + lo + 128],
                                        trib_sb)
                        pend.append((ki, half, lo, esb))
                        if len(pend) > 3:
                            emit_z(pend.pop(0))
                    if mid is not None and ki == kmax // 2:
                        mid()
                        mid = None
                for item in pend:
                    emit_z(item)
                # evacuate z + denominator rows to SBUF (frees psz banks)
                za = []
                for hh in range(2):
                    z = ep.tile([DH + 1, 1024], BF16, tag="zaug", name="zaug",
                                bufs=4)
                    with nc.allow_low_precision(reason="bf16 attention"):
                        nc.vector.tensor_copy(z, psz[hh])
                    za.append(z)
                return th, c, za

            def norm_pair(st):
                # 1/d via exp(-ln d): same activation-table set as softmax exp
                th, c, za = st
                ser = []
                for hh in range(2):
                    sl = wkp.tile([1, 1024], F32, tag="serl", bufs=2)
                    nc.scalar.activation(sl, za[hh][DH:DH + 1, :],
                                         mybir.ActivationFunctionType.Ln)
                    se = wkp.tile([1, 1024], BF16, tag="sere", bufs=2)
                    nc.scalar.activation(se, sl,
                                         mybir.ActivationFunctionType.Exp,
                                         scale=-1.0)
                    ser.append(se)
                for half in range(2):
                    psb = pa.tile([128, 512], F32, tag="pa", name="psb")
                    for hh in range(2):
                        # col-tiled pair: head hh broadcast at out partition 64*hh
                        nc.tensor.matmul(
                            psb[64 * hh:64 * (hh + 1), :], ones_sb,
                            ser[hh][:, 512 * half:512 * (half + 1)],
                            start=True, stop=True)
                    with nc.allow_low_precision(reason="bf16 attention"):
                        for hh in range(2):
                            nc.vector.tensor_mul(
                                zt_sb[th][64 * hh:64 * (hh + 1),
                                          1024 * c + 512 * half:
                                          1024 * c + 512 * (half + 1)],
                                za[hh][0:DH, 512 * half:512 * (half + 1)],
                                psb[64 * hh:64 * (hh + 1), :])

            def th_a2a(th):
                # my shard j = my 2 heads' z^T for q cols [256j, 256j+256);
                # received slot p = peer p's 2 heads for my 256 q rows.
                for jj in range(N_CORES):
                    nc.sync.dma_start(
                        a2a_in[th][128 * jj:128 * (jj + 1), :],
                        zt_sb[th][:, 256 * jj:256 * (jj + 1)])
                nc.gpsimd.collective_compute(
                    "AllToAll", mybir.AluOpType.bypass,
                    replica_groups=[[0, 1, 2, 3, 4, 5, 6, 7]],
                    ins=[a2a_in[th].opt()], outs=[a2a_out[th].opt()])
                dst = ztf_e if th == 0 else ztf_o
                for p in range(N_CORES):
                    nc.sync.dma_start(
                        dst[p], a2a_out[th][128 * p:128 * (p + 1), :])

            # ---------------- phase emission ----------------
            for u in qk_units(0, 0):
                u()
            for u in qk_units(0, 1):
                u()
            for u in v_units(0):
                u()
            for u in v_units(1):
                u()
            for i in range(DM_T):
                nc.sync.dma_start(wo_sb[i], w_o[128 * i:128 * (i + 1), :])

            st00 = attn_pair(0, 0)
            # x-chunk-1 projections emitted between chunks: the scheduler
            # back-fills attention's ScalarE-paced PE stalls with them
            for u in qk_units(1, 0):
                u()
            for u in qk_units(1, 1):
                u()
            for u in v_units(2):
                u()
            for u in v_units(3):
                u()
            st01 = attn_pair(0, 1, mid=lambda: norm_pair(st00))
            norm_pair(st01)
            th_a2a(0)
            st10 = attn_pair(1, 0)
            for u in outproj_units(0):
                u()
            st11 = attn_pair(1, 1, mid=lambda: norm_pair(st10))
            norm_pair(st11)
            th_a2a(1)
            for u in outproj_units(1):
                u()

    nc.finalize()
    return nc


_NC = None


def _get_nc():
    global _NC
    if _NC is None:
        _NC = build_bass()
    return _NC


def make_in_maps(query_input, key_input, value_input, additive_attention_mask,
                 W_Q, W_K, W_V, W_O, b_Q, b_K, b_V, b_O):
    f = np.float32
    bf = ml_dtypes.bfloat16
    tri = np.where(
        np.arange(128, dtype=np.int64)[None, :]
        >= np.arange(128, dtype=np.int64)[:, None],
        f(0.0), f(MASK_VAL)).astype(f)
    bob = np.ascontiguousarray(np.broadcast_to(b_O.astype(f), (128, DM)))
    trib_host = np.where(
        np.arange(128, dtype=np.int64)[None, :]
        >= np.arange(128, dtype=np.int64)[:, None],
        1.0, 0.0).astype(ml_dtypes.bfloat16)
    wo = np.ascontiguousarray(W_O.astype(f).reshape(DM, DM)).astype(bf)
    in_maps = []
    for c in range(N_CORES):
        b, rk = c // GROUP, c % GROUP
        hs = slice(H_LOC * rk, H_LOC * (rk + 1))
        wq = np.ascontiguousarray(
            W_Q[hs].astype(f).transpose(1, 0, 2).reshape(DM, WCOL)).astype(bf)
        wk = np.ascontiguousarray(
            W_K[hs].astype(f).transpose(1, 0, 2).reshape(DM, WCOL)).astype(bf)
        wv = np.ascontiguousarray(
            W_V[hs].astype(f).transpose(1, 0, 2).reshape(DM, WCOL)).astype(bf)
        bvb = np.zeros((128, H_LOC * (DH + 1)), ml_dtypes.bfloat16)
        for h in range(H_LOC):
            bvb[:, (DH + 1) * h:(DH + 1) * h + DH] = b_V[H_LOC * rk + h].astype(f)
            bvb[:, (DH + 1) * h + DH] = 1.0
        in_maps.append({
            "xt_q": np.ascontiguousarray(query_input[b].astype(f).T).astype(bf),
            "xt_k": np.ascontiguousarray(key_input[b].astype(f).T).astype(bf),
            "xt_v": np.ascontiguousarray(value_input[b].astype(f).T).astype(bf),
            "w_q": wq, "w_k": wk, "w_v": wv, "w_o": wo,
            "bq": np.ascontiguousarray(b_Q[hs].astype(f).reshape(WCOL, 1)),
            "bk": np.ascontiguousarray(b_K[hs].astype(f).reshape(WCOL, 1)),
            "bvb": bvb, "bob": bob,
            "trib": trib_host,
            "ones64": np.ones((1, DH), ml_dtypes.bfloat16),
            "maskt": np.ascontiguousarray(
                additive_attention_mask[b, 0, 0].astype(f).reshape(S_T, 128).T),
            "tri": tri,
        })
    return in_maps


def assemble_output(results):
    out = np.empty((B, S, DM), np.float32)
    for c in range(N_CORES):
        out[0, 256 * c:256 * (c + 1), :] = results[c]["out"][:256]
        out[1, 256 * c:256 * (c + 1), :] = results[c]["out"][256:]
    return out


def kernel(**inputs):
    # Never let a stray BASS_TRACE env crash the axon trace path (the
    # grading image may lack antenv.axon_hooks).
    os.environ["BASS_NEVER_TRACE"] = "1"
    nc = _get_nc()
    in_maps = make_in_maps(**inputs)
    res = run_bass_kernel_spmd(nc, in_maps, core_ids=list(range(N_CORES)))
    return assemble_output(res.results)


# revision 11
# speedup vs baseline: 1.1493x; 1.1493x over previous
"""Distributed Trainium2 Bass kernel for a full causal attention layer.

Problem: B=2, S=2048, D_MODEL=1024, H=16, D_HEAD=64, causal + additive mask.

Sharding (8 cores): data-parallel over batch (cores 0-3 -> batch 0,
cores 4-7 -> batch 1) x tensor-parallel over heads (4 heads per core).

v2 changes vs v1 (261984 ns):
  * Head-PAIR attention chunks: score matmuls for heads {2th, 2th+1} are
    row-tiled (K=64 at partitions 0-63 / 64-127) and run CONCURRENTLY on
    the PE's 32x32 subarrays -> ~halves score matmul time.
  * One exp per (ki, q-half) covers BOTH heads via a 3D AP over a shared
    [128, 1024] PSUM pair tile -> fewer ACTIVATE fixed overheads.
  * Softmax 1/d computed as exp(-ln d) with a manually preloaded combined
    activation-table set (natural_log_exp_and_others, id 6) -> exactly ONE
    ACT_TABLE_LOAD in the whole kernel (v1 alternated exp<->reciprocal
    sets: 12 loads x ~1.3us each, stalling ScalarE and starving the PE).
  * Norm broadcast matmuls col-tiled 2-up (output partitions 0/64 of one
    PSUM bank), norms for the previous chunk emitted MID-chunk so their
    ScalarE work hides inside the attention exp stream.
  * th-major chunk order: heads 0,1 finish at ~50% and their AllToAll
    fires then; projection filler units are pumped one-per-ki between
    attention steps (emission order = engine queue order, so pumping is
    windowed to avoid head-of-line blocking on not-yet-landed data).
  * PSUM: pss pair tile 2 banks + psz 2x2 banks + pa pool 2x1 = 8 exact.
Host only transposes/shards inputs and concatenates the 8 output slices.
"""

import os
import sys

import ml_dtypes
import numpy as np

for _p in ("/opt/trn_rl_repo", "/root/.axon_site/_ro/trn_rl_repo"):
    if os.path.isdir(_p) and _p not in sys.path:
        sys.path.insert(0, _p)

import concourse.bass as bass  # noqa: E402
import concourse.mybir as mybir  # noqa: E402
from concourse import bacc  # noqa: E402
from concourse import tile  # noqa: E402
from concourse.bass_utils import run_bass_kernel_spmd  # noqa: E402

F32 = mybir.dt.float32
BF16 = mybir.dt.bfloat16

B, S, DM, H, DH = 2, 2048, 1024, 16, 64
N_CORES = 8
GROUP = 4              # cores per batch group
H_LOC = H // GROUP     # heads per core
WCOL = H_LOC * DH      # 256 projected cols per core
QR = S // GROUP        # 512 q rows owned per core after AllToAll
MASK_VAL = -1.0e5
SCALE = 1.0 / np.sqrt(DH).astype(np.float32)

DM_T = DM // 128       # 8 dmodel k-tiles
S_T = S // 128         # 16 seq 128-tiles
ACT_SET_LN_EXP = 6     # natural_log_exp_and_others in act_info.json


def build_bass():
    nc = bacc.Bacc("TRN2", target_bir_lowering=False, debug=False,
                   num_devices=N_CORES)

    xt_q = nc.dram_tensor("xt_q", [DM, S], BF16, kind="ExternalInput")
    xt_k = nc.dram_tensor("xt_k", [DM, S], BF16, kind="ExternalInput")
    xt_v = nc.dram_tensor("xt_v", [DM, S], BF16, kind="ExternalInput")
    w_q = nc.dram_tensor("w_q", [DM, WCOL], BF16, kind="ExternalInput")
    w_k = nc.dram_tensor("w_k", [DM, WCOL], BF16, kind="ExternalInput")
    w_v = nc.dram_tensor("w_v", [DM, WCOL], BF16, kind="ExternalInput")
    w_o = nc.dram_tensor("w_o", [DM, DM], BF16, kind="ExternalInput")
    bq = nc.dram_tensor("bq", [WCOL, 1], F32, kind="ExternalInput")
    bk = nc.dram_tensor("bk", [WCOL, 1], F32, kind="ExternalInput")
    bvb = nc.dram_tensor("bvb", [128, H_LOC * (DH + 1)], BF16, kind="ExternalInput")
    bob = nc.dram_tensor("bob", [128, DM], F32, kind="ExternalInput")
    maskt = nc.dram_tensor("maskt", [128, S_T], F32, kind="ExternalInput")
    tri = nc.dram_tensor("tri", [128, 128], F32, kind="ExternalInput")
    trib = nc.dram_tensor("trib", [128, 128], BF16, kind="ExternalInput")
    ones64 = nc.dram_tensor("ones64", [1, DH], BF16, kind="ExternalInput")
    out = nc.dram_tensor("out", [QR, DM], F32, kind="ExternalOutput")

    # single activation-table load, emitted BEFORE the TileContext so the
    # Tile scheduler never sees it (it wedges the scheduling sim) but it
    # still precedes every activation on the ACT queue: set 6 covers exp
    # AND ln, so the softmax exps and the ln/exp reciprocal never thrash
    # the activation-function table (v1: 12 ACT_TABLE_LOADs of ~1.3us).
    nc.scalar.add_instruction(mybir.InstLoadActFuncSet(
        name=nc.get_next_instruction_name(),
        act_func_set_id=ACT_SET_LN_EXP, ins=[], outs=[]))

    with tile.TileContext(nc) as tc:
        with (
            tc.tile_pool(name="persist", bufs=1) as pp,
            tc.tile_pool(name="xts", bufs=10) as xtp,
            tc.tile_pool(name="esb", bufs=8) as ep,
            tc.tile_pool(name="work", bufs=4) as wkp,
            tc.tile_pool(name="pa", bufs=2, space="PSUM") as pa,
            tc.tile_pool(name="pss", bufs=2, space="PSUM") as pssp,
            tc.tile_pool(name="psz", bufs=1, space="PSUM") as pszp,
            tc.tile_pool(name="dram", bufs=1, space="DRAM") as dp,
        ):
            # ---- persistent SBUF tiles ----
            wq_sb = [pp.tile([128, WCOL], BF16, tag=f"wq{i}", name=f"wq{i}") for i in range(DM_T)]
            wk_sb = [pp.tile([128, WCOL], BF16, tag=f"wk{i}", name=f"wk{i}") for i in range(DM_T)]
            wv_sb = [pp.tile([128, WCOL], BF16, tag=f"wv{i}", name=f"wv{i}") for i in range(DM_T)]
            wo_sb = [pp.tile([128, DM], BF16, tag=f"wo{i}", name=f"wo{i}") for i in range(DM_T)]
            qt_sb = [pp.tile([128, S], BF16, tag=f"qt{t}", name=f"qt{t}") for t in range(2)]
            kt_sb = [pp.tile([128, S], BF16, tag=f"kt{t}", name=f"kt{t}") for t in range(2)]
            vaug = [pp.tile([128, H_LOC * (DH + 1)], BF16, tag=f"va{k}", name=f"va{k}")
                    for k in range(S_T)]
            zt_sb = [pp.tile([128, S], BF16, tag=f"zt{t}", name=f"zt{t}") for t in range(2)]
            ztf_e = [pp.tile([128, 256], BF16, tag=f"zfe{i}", name=f"zfe{i}")
                     for i in range(N_CORES)]
            ztf_o = [pp.tile([128, 256], BF16, tag=f"zfo{i}", name=f"zfo{i}")
                     for i in range(N_CORES)]
            bq_sb = [pp.tile([128, 1], F32, tag=f"bq{t}", name=f"bq{t}") for t in range(2)]
            bk_sb = [pp.tile([128, 1], F32, tag=f"bk{t}", name=f"bk{t}") for t in range(2)]
            bvb_sb = pp.tile([128, H_LOC * (DH + 1)], BF16, tag="bvb")
            bob_sb = pp.tile([128, DM], F32, tag="bob")
            maskt_sb = pp.tile([128, S_T], F32, tag="maskt")
            trib_sb = pp.tile([128, 128], BF16, tag="trib")
            ones_sb = pp.tile([1, DH], BF16, tag="ones")
            oacc = [pp.tile([128, DM], F32, tag=f"oacc{i}", name=f"oacc{i}")
                    for i in range(4)]
            a2a_in = [dp.tile([N_CORES * 128, 256], BF16, tag=f"a2a_in{t}",
                              name=f"a2a_in{t}") for t in range(2)]
            a2a_out = [dp.tile([N_CORES * 128, 256], BF16, tag=f"a2a_out{t}",
                               name=f"a2a_out{t}") for t in range(2)]

            # ---- constants ----
            for t in range(2):
                nc.sync.dma_start(bq_sb[t], bq[128 * t:128 * (t + 1), :])
                nc.sync.dma_start(bk_sb[t], bk[128 * t:128 * (t + 1), :])
            nc.sync.dma_start(bvb_sb, bvb[:, :])
            nc.sync.dma_start(bob_sb, bob[:, :])
            nc.sync.dma_start(maskt_sb, maskt[:, :])
            nc.sync.dma_start(trib_sb, trib[:, :])
            nc.sync.dma_start(ones_sb, ones64[:, :])

            # ---------------- projection units (filler pump) ----------------
            def qk_units(xc, which):
                # which: 0 -> Q, 1 -> K. One unit per (wc, hf): an 8-deep dm
                # accumulation chain of N=512 matmuls + bias add into qt/kt.
                src_dram = (xt_q, xt_k)[which]
                w_dram = (w_q, w_k)[which]
                w_t = (wq_sb, wk_sb)[which]
                b_t = (bq_sb, bk_sb)[which]
                dst = (qt_sb, kt_sb)[which]
                xx = [None] * DM_T

                def load_x():
                    for dm in range(DM_T):
                        xx[dm] = xtp.tile([128, 1024], BF16, tag="xq", name="xq")
                        nc.sync.dma_start(
                            xx[dm],
                            src_dram[128 * dm:128 * (dm + 1),
                                     1024 * xc:1024 * (xc + 1)])
                        if xc == 0:
                            nc.sync.dma_start(
                                w_t[dm], w_dram[128 * dm:128 * (dm + 1), :])

                units = []
                for wc in range(2):
                    for hf in range(2):
                        def u(wc=wc, hf=hf):
                            if xx[0] is None:
                                load_x()
                            pq = pa.tile([128, 512], F32, tag="pa", name="pq")
                            for dm in range(DM_T):
                                nc.tensor.matmul(
                                    pq,
                                    w_t[dm][:, 128 * wc:128 * (wc + 1)],
                                    xx[dm][:, 512 * hf:512 * (hf + 1)],
                                    start=(dm == 0), stop=(dm == DM_T - 1))
                            with nc.allow_low_precision(reason="bf16 attention"):
                                nc.vector.tensor_scalar_add(
                                    dst[wc][:, 1024 * xc + 512 * hf:
                                            1024 * xc + 512 * (hf + 1)],
                                    pq, b_t[wc])
                        units.append(u)
                return units

            def v_units(xc):
                # one unit per 128-seq block: 8-deep dm chain, N=256, then
                # bias-add + ones-column into the augmented V tile
                xv_t = [None] * DM_T

                def load_x():
                    for dm in range(DM_T):
                        xv_t[dm] = xtp.tile([128, 512], BF16, tag="xv", name="xv")
                        nc.sync.dma_start(
                            xv_t[dm],
                            xt_v[128 * dm:128 * (dm + 1),
                                 512 * xc:512 * (xc + 1)])
                        if xc == 0:
                            nc.sync.dma_start(
                                wv_sb[dm], w_v[128 * dm:128 * (dm + 1), :])

                units = []
                for blk in range(4):
                    def u(blk=blk):
                        if xv_t[0] is None:
                            load_x()
                        ki = 4 * xc + blk
                        psv = pa.tile([128, 512], F32, tag="pa", name="pav")
                        for dm in range(DM_T):
                            nc.tensor.matmul(
                                psv[:, 0:WCOL],
                                xv_t[dm][:, 128 * blk:128 * (blk + 1)],
                                wv_sb[dm], start=(dm == 0), stop=(dm == DM_T - 1))
                        va3 = vaug[ki].rearrange("p (h x) -> p h x", h=H_LOC)
                        bvb3 = bvb_sb.rearrange("p (h x) -> p h x", h=H_LOC)
                        psv3 = psv[:, 0:WCOL].rearrange("p (h d) -> p h d", h=H_LOC)
                        with nc.allow_low_precision(reason="bf16 attention"):
                            nc.vector.scalar_tensor_tensor(
                                va3[:, :, 0:DH], psv3, 1.0, bvb3[:, :, 0:DH],
                                op0=mybir.AluOpType.mult, op1=mybir.AluOpType.add)
                            nc.vector.tensor_copy(
                                va3[:, :, DH:DH + 1], bvb3[:, :, DH:DH + 1])
                    units.append(u)
                return units

            def outproj_units(parity):
                # parity 0: even head-pairs (ztf_e, wo tiles 2g): accumulate
                # into oacc with bias; parity 1: odds: combine + store.
                ztf = ztf_e if parity == 0 else ztf_o
                units = []
                for bh in range(2):
                    for qt in range(2):
                        for hf in range(2):
                            def u(bh=bh, qt=qt, hf=hf):
                                pso = pa.tile([128, 512], F32, tag="pa", name="pso")
                                for g in range(4):
                                    nc.tensor.matmul(
                                        pso,
                                        ztf[4 * bh + g][:, 128 * qt:128 * (qt + 1)],
                                        wo_sb[2 * g + parity][:, 512 * hf:512 * (hf + 1)],
                                        start=(g == 0), stop=(g == 3))
                                acc = oacc[2 * bh + qt]
                                sl = slice(512 * hf, 512 * (hf + 1))
                                if parity == 0:
                                    nc.vector.tensor_add(
                                        acc[:, sl], pso, bob_sb[:, sl])
                                else:
                                    osb = wkp.tile([128, 512], F32, tag="osb")
                                    nc.vector.tensor_add(osb, pso, acc[:, sl])
                                    nc.sync.dma_start(
                                        out[256 * bh + 128 * qt:
                                            256 * bh + 128 * (qt + 1), sl],
                                        osb)
                            units.append(u)
                return units

            # ---------------- attention ----------------
            za_cur = [None, None]

            def attn_chunk(th, c5, mid=None):
                """Causal attention for heads {2th, 2th+1}, 512-wide q chunk
                c5, scores transposed [k, q].  The two heads' score matmuls
                are row-tiled (K=64 at partitions 0/64) and run concurrently
                into one double-buffered [128, 1024] PSUM pair tile; one exp
                per ki covers both heads via a 3D AP.  Evacuated z halves
                accumulate into a [65, 1024] SBUF tile shared by chunk pairs
                so norms stay at 1024 granularity."""
                kmax = 4 * c5 + 4
                qb = 512 * c5
                psz = [pszp.tile([DH + 1, 512], F32, tag=f"psz{hh}",
                                 name=f"psz{hh}") for hh in range(2)]
                pend = []

                def emit_z(item):
                    ki, lo, esb = item
                    for hh in range(2):
                        nc.tensor.matmul(
                            psz[hh][:, lo:512],
                            vaug[ki][:, (DH + 1) * (2 * th + hh):
                                     (DH + 1) * (2 * th + hh + 1)],
                            esb[:, 512 * hh + lo:512 * (hh + 1)],
                            start=(ki == 0), stop=(ki == kmax - 1))

                for ki in range(kmax):
                    dcol = 128 * ki - qb
                    lo = max(0, dcol)
                    pss = pssp.tile([128, 1024], F32, tag="pss", name="pss")
                    for hh in range(2):
                        nc.tensor.matmul(
                            pss[:, 512 * hh + lo:512 * (hh + 1)],
                            kt_sb[th][64 * hh:64 * (hh + 1),
                                      128 * ki:128 * (ki + 1)],
                            qt_sb[th][64 * hh:64 * (hh + 1),
                                      qb + lo:qb + 512],
                            start=True, stop=True)
                    esb = ep.tile([128, 1024], BF16, tag="e", name="esb")
                    p3 = pss.rearrange("p (h w) -> p h w", h=2)
                    e3 = esb.rearrange("p (h w) -> p h w", h=2)
                    nc.scalar.activation(
                        e3[:, :, lo:512], p3[:, :, lo:512],
                        mybir.ActivationFunctionType.Exp,
                        bias=maskt_sb[:, ki:ki + 1], scale=float(SCALE))
                    if dcol >= 0:
                        # diagonal: post-exp 0/1 triangle mask per head
                        with nc.allow_low_precision(reason="bf16 attention"):
                            for hh in range(2):
                                nc.vector.tensor_mul(
                                    esb[:, 512 * hh + lo:512 * hh + lo + 128],
                                    esb[:, 512 * hh + lo:512 * hh + lo + 128],
                                    trib_sb)
                    pend.append((ki, lo, esb))
                    if len(pend) > 2:
                        emit_z(pend.pop(0))
                    if mid is not None and ki == kmax // 2:
                        mid()
                        mid = None
                for item in pend:
                    emit_z(item)
                # evacuate z + denominator rows into the chunk-pair SBUF tile
                for hh in range(2):
                    if c5 % 2 == 0:
                        za_cur[hh] = ep.tile([DH + 1, 1024], BF16, tag="zaug",
                                             name="zaug", bufs=4)
                    with nc.allow_low_precision(reason="bf16 attention"):
                        nc.vector.tensor_copy(
                            za_cur[hh][:, 512 * (c5 % 2):512 * (c5 % 2 + 1)],
                            psz[hh])
                return th, c5 // 2, list(za_cur)

            def norm_pair(st):
                # 1/d via exp(-ln d): same activation-table set as softmax exp
                th, c, za = st
                ser = []
                for hh in range(2):
                    sl = wkp.tile([1, 1024], F32, tag="serl", bufs=2)
                    nc.scalar.activation(sl, za[hh][DH:DH + 1, :],
                                         mybir.ActivationFunctionType.Ln)
                    se = wkp.tile([1, 1024], BF16, tag="sere", bufs=2)
                    nc.scalar.activation(se, sl,
                                         mybir.ActivationFunctionType.Exp,
                                         scale=-1.0)
                    ser.append(se)
                for half in range(2):
                    psb = pa.tile([128, 512], F32, tag="pa", name="psb")
                    for hh in range(2):
                        # col-tiled pair: head hh broadcast at out partition 64*hh
                        nc.tensor.matmul(
                            psb[64 * hh:64 * (hh + 1), :], ones_sb,
                            ser[hh][:, 512 * half:512 * (half + 1)],
                            start=True, stop=True)
                    with nc.allow_low_precision(reason="bf16 attention"):
                        for hh in range(2):
                            nc.vector.tensor_mul(
                                zt_sb[th][64 * hh:64 * (hh + 1),
                                          1024 * c + 512 * half:
                                          1024 * c + 512 * (half + 1)],
                                za[hh][0:DH, 512 * half:512 * (half + 1)],
                                psb[64 * hh:64 * (hh + 1), :])

            def th_a2a(th):
                # my shard j = my 2 heads' z^T for q cols [256j, 256j+256);
                # received slot p = peer p's 2 heads for my 256 q rows.
                for jj in range(N_CORES):
                    nc.sync.dma_start(
                        a2a_in[th][128 * jj:128 * (jj + 1), :],
                        zt_sb[th][:, 256 * jj:256 * (jj + 1)])
                nc.gpsimd.collective_compute(
                    "AllToAll", mybir.AluOpType.bypass,
                    replica_groups=[[0, 1, 2, 3, 4, 5, 6, 7]],
                    ins=[a2a_in[th].opt()], outs=[a2a_out[th].opt()])
                dst = ztf_e if th == 0 else ztf_o
                for p in range(N_CORES):
                    nc.sync.dma_start(
                        dst[p], a2a_out[th][128 * p:128 * (p + 1), :])

            # ---------------- phase emission ----------------
            for u in qk_units(0, 0):
                u()
            for u in qk_units(0, 1):
                u()
            for u in v_units(0):
                u()
            for u in v_units(1):
                u()
            for i in range(DM_T):
                nc.sync.dma_start(wo_sb[i], w_o[128 * i:128 * (i + 1), :])

            attn_chunk(0, 0)
            st_a = attn_chunk(0, 1)
            # x-chunk-1 projections emitted between chunks: the scheduler
            # back-fills attention's ScalarE-paced PE stalls with them
            for u in qk_units(1, 0):
                u()
            for u in qk_units(1, 1):
                u()
            for u in v_units(2):
                u()
            for u in v_units(3):
                u()
            attn_chunk(0, 2, mid=lambda: norm_pair(st_a))
            st_b = attn_chunk(0, 3)
            norm_pair(st_b)
            th_a2a(0)
            attn_chunk(1, 0)
            st_c = attn_chunk(1, 1)
            attn_chunk(1, 2, mid=lambda: norm_pair(st_c))
            # evens: emitted late enough that a2a(0) has landed (avoids
            # head-of-line blocking the PE queue on ztf_e)
            for u in outproj_units(0):
                u()
            st_d = attn_chunk(1, 3)
            norm_pair(st_d)
            th_a2a(1)
            for u in outproj_units(1):
                u()

    nc.finalize()
    return nc


_NC = None


def _get_nc():
    global _NC
    if _NC is None:
        _NC = build_bass()
    return _NC


def make_in_maps(query_input, key_input, value_input, additive_attention_mask,
                 W_Q, W_K, W_V, W_O, b_Q, b_K, b_V, b_O):
    f = np.float32
    bf = ml_dtypes.bfloat16
    tri = np.where(
        np.arange(128, dtype=np.int64)[None, :]
        >= np.arange(128, dtype=np.int64)[:, None],
        f(0.0), f(MASK_VAL)).astype(f)
    bob = np.ascontiguousarray(np.broadcast_to(b_O.astype(f), (128, DM)))
    trib_host = np.where(
        np.arange(128, dtype=np.int64)[None, :]
        >= np.arange(128, dtype=np.int64)[:, None],
        1.0, 0.0).astype(ml_dtypes.bfloat16)
    wo = np.ascontiguousarray(W_O.astype(f).reshape(DM, DM)).astype(bf)
    in_maps = []
    for c in range(N_CORES):
        b, rk = c // GROUP, c % GROUP
        hs = slice(H_LOC * rk, H_LOC * (rk + 1))
        wq = np.ascontiguousarray(
            W_Q[hs].astype(f).transpose(1, 0, 2).reshape(DM, WCOL)).astype(bf)
        wk = np.ascontiguousarray(
            W_K[hs].astype(f).transpose(1, 0, 2).reshape(DM, WCOL)).astype(bf)
        wv = np.ascontiguousarray(
            W_V[hs].astype(f).transpose(1, 0, 2).reshape(DM, WCOL)).astype(bf)
        bvb = np.zeros((128, H_LOC * (DH + 1)), ml_dtypes.bfloat16)
        for h in range(H_LOC):
            bvb[:, (DH + 1) * h:(DH + 1) * h + DH] = b_V[H_LOC * rk + h].astype(f)
            bvb[:, (DH + 1) * h + DH] = 1.0
        in_maps.append({
            "xt_q": np.ascontiguousarray(query_input[b].astype(f).T).astype(bf),
            "xt_k": np.ascontiguousarray(key_input[b].astype(f).T).astype(bf),
            "xt_v": np.ascontiguousarray(value_input[b].astype(f).T).astype(bf),
            "w_q": wq, "w_k": wk, "w_v": wv, "w_o": wo,
            "bq": np.ascontiguousarray(b_Q[hs].astype(f).reshape(WCOL, 1)),
            "bk": np.ascontiguousarray(b_K[hs].astype(f).reshape(WCOL, 1)),
            "bvb": bvb, "bob": bob,
            "trib": trib_host,
            "ones64": np.ones((1, DH), ml_dtypes.bfloat16),
            "maskt": np.ascontiguousarray(
                additive_attention_mask[b, 0, 0].astype(f).reshape(S_T, 128).T),
            "tri": tri,
        })
    return in_maps


def assemble_output(results):
    out = np.empty((B, S, DM), np.float32)
    for c in range(N_CORES):
        out[0, 256 * c:256 * (c + 1), :] = results[c]["out"][:256]
        out[1, 256 * c:256 * (c + 1), :] = results[c]["out"][256:]
    return out


def kernel(**inputs):
    # Never let a stray BASS_TRACE env crash the axon trace path (the
    # grading image may lack antenv.axon_hooks).
    os.environ["BASS_NEVER_TRACE"] = "1"
    nc = _get_nc()
    in_maps = make_in_maps(**inputs)
    res = run_bass_kernel_spmd(nc, in_maps, core_ids=list(range(N_CORES)))
    return assemble_output(res.results)
